# revision 41
# baseline (speedup 1.0000x reference)
"""Trainium2 Bass kernel for nn_MultiHeadAttention_45037027065972.

Head-parallel sharding: the reference's reshape `(B,S,H*D) -> (B,H,S,D)`
means head h of batch b only reads rows [128h, 128h+128) of the projection
inputs.  32 (b,h) slices are sharded 4-per-core across 8 cores (cores 0-3:
batch 0, cores 4-7: batch 1).  Each core projects its 4 slabs, runs full
S x S causal attention per slice in a transposed (k-major) layout, folds
the per-head output projection, and emits a per-core partial of
`sum_h out_h @ Wo_h` (shape [e=128, q=2048]).  The host unshard sums the
4 partials per batch, transposes, and adds bo.

v3 over the baseline:
  - ragged diagonal blocks: the k-chunk matmuls of the diagonal 512x512
    block only cover q >= 128j (scores, AV, lB and exp all skip ~15% of
    the attention work); causal masking shrinks to two [128,2,128]
    affine_selects per slice-panel on GPSIMD.
  - the Q^T/K^T slab eviction scatters two m-columns per call (the two
    projection matmuls share one PSUM tile), so the strided writes come
    in contiguous 8-byte pairs; the per-m bias rides along via a
    stride-0 broadcast AP.  (The biases do NOT cancel in softmax: the
    reshape makes them depend on s mod 16.)
  - ACT does nothing but exp during the attention phase; V-projection
    PSUM->SBUF copies and all epilogue work live on DVE, whose total
    load stays well under the PE's.
  - units run panels (3,sl),(2,sl) across slices, then (0,sl),(1,sl), so
    only two Wo accumulators are ever live; the V pipeline for slice
    sl+1 is interleaved right after the first unit touching slice sl.

Attention (S=2048, D=128), matmul operands bf16 (PSUM accumulate fp32):
  per slice scoresT[k,q] tiles = (K^T chunk stationary) @ (Q^T panel moving)
  P~ = exp(scoresT/sqrt(D)) on ACT (scores in [-9,9]: no running max);
  oT[d,q]  += V-chunk @ P~        (PSUM accumulation over k chunks)
  lB[*,q]  += ones128 @ P~        (row-sum broadcast across partitions)
  rb = 1/lB via DVE reciprocal_approx_fast; osbn = oT * rb (bf16)
  acc[e,q] += Wo_h^T @ osbn       (PSUM accumulation across the 4 slices)
"""

import sys
import math
from collections import deque

import numpy as np

for _p in ("/opt/trn_rl_repo", "/opt/pypackages"):
    if _p not in sys.path:
        sys.path.append(_p)

import ml_dtypes
import concourse.bacc as bacc
import concourse.mybir as mybir
import concourse.tile as tile
from concourse.bass_utils import run_bass_kernel_spmd

B, S, H, D = 2, 2048, 16, 128
NCORES = 8
NSLICE = 4            # (b,h) slices per core
PANEL = 512           # q panel width
NPANEL = S // PANEL   # 4
SCALE = 1.0 / math.sqrt(128.0)
F32 = mybir.dt.float32
BF16 = mybir.dt.bfloat16
F32R = mybir.dt.float32r
AF = mybir.ActivationFunctionType
ALU = mybir.AluOpType
BF_NP = ml_dtypes.bfloat16

_CACHE = {}
_ONES = np.ones((128, 128), BF_NP)


def _build():
    nc = bacc.Bacc(trn_type="TRN2", target_bir_lowering=False, debug=False)

    qT_d = nc.dram_tensor("qT", [128, NSLICE * 128], BF16, kind="ExternalInput")
    kT_d = nc.dram_tensor("kT", [128, NSLICE * 128], BF16, kind="ExternalInput")
    vT_d = nc.dram_tensor("vT", [128, NSLICE * 128], BF16, kind="ExternalInput")
    Wq_d = nc.dram_tensor("Wq", [128, 2048], BF16, kind="ExternalInput")
    Wk_d = nc.dram_tensor("Wk", [128, 2048], BF16, kind="ExternalInput")
    Wv_d = nc.dram_tensor("Wv", [128, 2048], BF16, kind="ExternalInput")
    Wo4_d = nc.dram_tensor("Wo4", [128, NSLICE * 128], BF16, kind="ExternalInput")
    bqT_d = nc.dram_tensor("bqT", [128, 16], BF16, kind="ExternalInput")
    bkT_d = nc.dram_tensor("bkT", [128, 16], BF16, kind="ExternalInput")
    bv_d = nc.dram_tensor("bv_bc", [128, 2048], BF16, kind="ExternalInput")
    ones_d = nc.dram_tensor("ones", [128, 128], BF16, kind="ExternalInput")
    out_d = nc.dram_tensor("partial", [128, S], BF16, kind="ExternalOutput")
    vscr_d = nc.dram_tensor("vscratch", [NSLICE, 128, 2048], BF16)

    with tile.TileContext(nc) as tc:
        with (
            tc.tile_pool(name="const", bufs=1) as const,
            tc.tile_pool(name="slab", bufs=1) as slab,
            tc.tile_pool(name="vslp", bufs=2) as vslp,
            tc.tile_pool(name="pbp", bufs=5) as pbp,
            tc.tile_pool(name="osbp", bufs=2) as osbp,
            tc.tile_pool(name="rbp", bufs=2) as rbp,
            tc.tile_pool(name="accp", bufs=2) as accp,
            tc.tile_pool(name="stgp", bufs=1) as stgp,
            tc.tile_pool(name="psS", bufs=2, space="PSUM") as psS,
            tc.tile_pool(name="psA", bufs=2, space="PSUM") as psA,
            tc.tile_pool(name="psO", bufs=2, space="PSUM") as psO,
        ):
            # ---- resident constants; DMA order = first-use order ----
            Wsb = {}
            xT = {}
            xdr = {"v": vT_d, "q": qT_d, "k": kT_d}
            wdr = {"v": Wv_d, "q": Wq_d, "k": Wk_d}
            for name in ("v", "q", "k"):
                t = const.tile([128, NSLICE * 128], BF16, tag=f"x{name}")
                xT[name] = t
                w = const.tile([128, 2048], BF16, tag=f"W{name}")
                Wsb[name] = w
            bqT = const.tile([128, 16], BF16, tag="bqT")
            bkT = const.tile([128, 16], BF16, tag="bkT")
            biasTT = {"q": bqT, "k": bkT}
            ones_sb = const.tile([128, 128], BF16, tag="ones")
            bv_sb = const.tile([128, 2048], BF16, tag="bv")
            wo4 = const.tile([128, NSLICE * 128], BF16, tag="wo4")
            dummy = const.tile([1, 16], F32, tag="dummy")

            # Force the exp table set to load NOW, while ACT is idle (the
            # Identity used by the ACT scatter pairs lives in every set, so
            # no second table switch before the first attention exp).
            nc.vector.memset(dummy[:], 1.0)
            nc.scalar.activation(dummy[:], dummy[:], AF.Exp)

            # Phase-1 DMAs (sync/gpsimd only: ACT must stay exp-only and
            # DVE is the scatter engine): V-pipeline inputs first (slice-0
            # V projection runs on the PE before the Q/K projection).
            nc.gpsimd.dma_start(out=Wsb["v"][:, :512], in_=wdr["v"][:, :512])
            nc.sync.dma_start(out=xT["v"][:], in_=xdr["v"][:])
            nc.sync.dma_start(out=Wsb["v"][:, 512:], in_=wdr["v"][:, 512:])
            nc.sync.dma_start(out=bv_sb[:], in_=bv_d[:])
            nc.gpsimd.dma_start(out=xT["k"][:], in_=xdr["k"][:])
            nc.sync.dma_start(out=xT["q"][:], in_=xdr["q"][:])
            nc.gpsimd.dma_start(out=Wsb["k"][:], in_=wdr["k"][:])
            nc.sync.dma_start(out=Wsb["q"][:], in_=wdr["q"][:])
            nc.gpsimd.dma_start(out=bqT[:], in_=bqT_d[:])
            nc.sync.dma_start(out=bkT[:], in_=bkT_d[:])
            nc.gpsimd.dma_start(out=ones_sb[:], in_=ones_d[:])
            nc.sync.dma_start(out=wo4[:], in_=Wo4_d[:])

            vch = slab.tile([128, NSLICE * 2048], BF16, tag="vch")

            def v_proj_slice(sl):
                # V projection for one slice: two [128,1024] PSUM tiles of
                # two 512-col matmuls each; the bias rides the DVE eviction
                # (STT add with the host-broadcast bv tile).  Then the DRAM
                # bounce into chunk layout
                # vch[16u+w, (i,d)] = vsl[8i+u, 128w+d] (dest [128,128]
                # iterates (16u+w, d) exactly as source [8,16,128] does).
                vsl = vslp.tile([128, 2048], BF16, tag="vsl")
                if sl == 0:
                    # slice 0 runs before attention: use the psA ring (its
                    # oT/lB tiles are first allocated mid-attention) so the
                    # psS ring stays free for the first projection tiles
                    for qtr in range(4):
                        vq = psA.tile([128, 512], F32, tag="av", name="vq")
                        nc.tensor.matmul(
                            vq[:],
                            lhsT=xT["v"][:, sl * 128:(sl + 1) * 128],
                            rhs=Wsb["v"][:, qtr * 512:(qtr + 1) * 512],
                            start=True, stop=True,
                        )
                        nc.vector.scalar_tensor_tensor(
                            vsl[:, qtr * 512:(qtr + 1) * 512], vq[:], 1.0,
                            bv_sb[:, qtr * 512:(qtr + 1) * 512],
                            op0=ALU.mult, op1=ALU.add)
                else:
                    for hf in range(2):
                        vq = psS.tile([128, 1024], F32, tag="sc", name="vq")
                        for qtr in (2 * hf, 2 * hf + 1):
                            nc.tensor.matmul(
                                vq[:, (qtr % 2) * 512:(qtr % 2) * 512 + 512],
                                lhsT=xT["v"][:, sl * 128:(sl + 1) * 128],
                                rhs=Wsb["v"][:, qtr * 512:(qtr + 1) * 512],
                                start=True, stop=True,
                            )
                        nc.vector.scalar_tensor_tensor(
                            vsl[:, hf * 1024:(hf + 1) * 1024], vq[:], 1.0,
                            bv_sb[:, hf * 1024:(hf + 1) * 1024],
                            op0=ALU.mult, op1=ALU.add)
                nc.sync.dma_start(out=vscr_d[sl], in_=vsl[:])
                nc.sync.dma_start(
                    out=vch[:, sl * 2048:(sl + 1) * 2048].rearrange(
                        "p (i d) -> p i d", i=16),
                    in_=vscr_d[sl].rearrange(
                        "(i u) (w d) -> (u w) i d", u=8, w=16),
                )

            v_proj_slice(0)

            # Q^T / K^T slabs in s' order: col (s, 16j + m).  Two-stage
            # eviction: ACT (idle pre-attention) copies each projection
            # m-pair PSUM tile to a persistent bf16 SBUF staging tile
            # (contiguous, fast); DVE then scatters m-pairs (contiguous
            # 4-byte write pairs at stride 32B) with the per-m bias via a
            # stride-0 broadcast AP.  Slices {0,1} scatter immediately so
            # attention can start as soon as the ACT stream drains; the
            # slice {2,3} scatters are deferred into the first attention
            # units (the staging tiles persist, so no PSUM lifetime issue).
            QKp = {}
            for name in ("q", "k"):
                dst = slab.tile([128, NSLICE * 2048], BF16, tag=f"{name}T")
                QKp[name] = dst
            late_scatter = []

            def mk_scatter(name, stg, mp, lo, hi):
                def go():
                    dst = QKp[name]
                    dview = dst[:].rearrange(
                        "p (sj w) -> p sj w",
                        w=16)[:, lo:hi, 2 * mp:2 * mp + 2]
                    src = stg[:].rearrange(
                        "p (w sj) -> p sj w", w=2)[:, lo:hi, :]
                    bias_bc = biasTT[name][:, 2 * mp:2 * mp + 2].rearrange(
                        "p (a w) -> p a w", a=1).broadcast_to(
                        [128, hi - lo, 2])
                    nc.vector.scalar_tensor_tensor(
                        dview, src, 1.0, bias_bc,
                        op0=ALU.mult, op1=ALU.add)
                return go

            for mp in range(8):
                for name in ("k", "q"):
                    pt = psS.tile([128, 1024], F32, tag="sc", name="pt")
                    for half in range(2):
                        m = 2 * mp + half
                        nc.tensor.matmul(
                            pt[:, half * 512:(half + 1) * 512],
                            lhsT=Wsb[name][:, m * 128:(m + 1) * 128],
                            rhs=xT[name][:],
                            start=True, stop=True,
                        )
                    stg = stgp.tile(
                        [128, 1024], BF16, tag=f"g{name}{mp}", name="stg")
                    nc.scalar.activation(stg[:], pt[:], AF.Copy)
                    mk_scatter(name, stg, mp, 0, 256)()
                    late_scatter.append(
                        mk_scatter(name, stg, mp, 256, 512))

            # ---- attention: panel-outer (desc), slice-inner ----
            # Deferred-emission queue: AV/lB for group g land two groups
            # later (hiding the exp + causal-mask latency), the epilogue
            # for a unit lands during the next unit's first groups.  PSUM
            # tiles are allocated lazily inside the deferred emitters so
            # pool rotation only ever reuses a tile whose readers are
            # already emitted.
            QT_all = QKp["q"]
            KT_all = QKp["k"]
            pending = deque()   # deferred emitters

            def flush(n):
                for _ in range(min(n, len(pending))):
                    pending.popleft()()

            def mk_avlb(parts, st, pb, VC, first, last, lb_first):
                # parts: list of (chunk_idx, pb_col_off, width, q_off).
                # lb_first alternates by group parity so the lB matmuls of
                # consecutive groups are adjacent and share the `ones`
                # stationary (one fewer LDWEIGHTS per group pair).
                def go():
                    if first:
                        st["oT"] = psA.tile([128, 512], F32, tag="av", name="oT")
                        st["lB"] = psA.tile([128, 512], F32, tag="av", name="lB")
                    oT, lB = st["oT"], st["lB"]
                    n = len(parts)

                    def av():
                        for pi, (ci, off, w, qo) in enumerate(parts):
                            nc.tensor.matmul(
                                oT[:, qo:512],
                                lhsT=VC[:, ci * 128:(ci + 1) * 128],
                                rhs=pb[:, off:off + w],
                                start=(first and pi == 0),
                                stop=(last and pi == n - 1),
                            )

                    def lb():
                        for pi, (ci, off, w, qo) in enumerate(parts):
                            nc.tensor.matmul(
                                lB[:, qo:512],
                                lhsT=ones_sb[:],
                                rhs=pb[:, off:off + w],
                                start=(first and pi == 0),
                                stop=(last and pi == n - 1),
                            )

                    if lb_first:
                        lb()
                        av()
                    else:
                        av()
                        lb()
                return go

            def mk_epiA(st):
                def go():
                    # normalize oT into bf16 osbn for the Wo fold
                    rb = rbp.tile([128, 512], F32, tag="rb")
                    nc.vector.reciprocal_approx_fast(
                        out=rb[:], in_=st["lB"][:])
                    osbn = osbp.tile([128, 512], BF16, tag="osbn")
                    nc.vector.tensor_tensor(
                        osbn[:], st["oT"][:], rb[:], ALU.mult)
                    st["osbn"] = osbn
                return go

            def mk_epiB(sl, p, st, pst):
                def go():
                    if sl == 0:
                        pst["acps"] = psO.tile([128, 512], F32, tag="acps", name="acps")
                    acps = pst["acps"]
                    nc.tensor.matmul(
                        acps[:],
                        lhsT=wo4[:, sl * 128:(sl + 1) * 128],
                        rhs=st["osbn"][:],
                        start=(sl == 0), stop=(sl == NSLICE - 1),
                    )
                    if sl == NSLICE - 1:
                        acc_sb = accp.tile([128, 512], BF16, tag="acc")
                        nc.vector.tensor_copy(acc_sb[:], acps[:])
                        nc.sync.dma_start(
                            out=out_d[:, p * 512:(p + 1) * 512],
                            in_=acc_sb[:])
                return go

            # unit order: the two long panels across all slices, then the
            # two short panels (only two Wo accumulators live at a time).
            units = []
            for sl in range(NSLICE):
                units += [(3, sl), (2, sl)]
            for sl in range(NSLICE):
                units += [(0, sl), (1, sl)]
            vins = {0: 1, 2: 2, 4: 3}  # after unit idx -> v_proj_slice(sl)
            psts = {p: {} for p in range(NPANEL)}
            for ui, (p, sl) in enumerate(units):
                pst = psts[p]
                VC = vch[:, sl * 2048:(sl + 1) * 2048]
                qt0 = sl * 2048 + p * 512
                st = {}
                # groups: full chunk pairs, then the ragged diagonal pairs.
                # Each entry: (sc_parts, mask_stride or None);
                # sc_parts: list of (chunk, sc_off, width, q_off)
                groups = []
                for g in range(2 * p):
                    groups.append((
                        [(2 * g, 0, 512, 0), (2 * g + 1, 512, 512, 0)],
                        None))
                groups.append((
                    [(4 * p, 0, 512, 0), (4 * p + 1, 512, 384, 128)],
                    512))
                groups.append((
                    [(4 * p + 2, 0, 256, 256), (4 * p + 3, 256, 128, 384)],
                    256))
                ng = len(groups)
                popped = 0
                for gi, (parts, mstride) in enumerate(groups):
                    sc = psS.tile([128, 1024], F32, tag="sc")
                    for ci, off, w, qo in parts:
                        nc.tensor.matmul(
                            sc[:, off:off + w],
                            lhsT=KT_all[:, sl * 2048 + ci * 128:
                                        sl * 2048 + (ci + 1) * 128],
                            rhs=QT_all[:, qt0 + qo:qt0 + 512],
                            start=True, stop=True,
                        )
                    # lag-3 software pipeline: consumers trail producers by
                    # three groups (pbp bufs=5 covers the pb lifetime)
                    n_pop = 1 if gi <= 1 else (2 if gi <= 3 else 1)
                    popped += n_pop
                    flush(n_pop)
                    width = parts[-1][1] + parts[-1][2]
                    pb = pbp.tile([128, 1024], BF16, tag="pb")
                    nc.scalar.activation(
                        pb[:, :width], sc[:, :width], AF.Exp, scale=SCALE)
                    if mstride is not None:
                        # zero where q < k in the two 128-wide triangle
                        # blocks at col 0 and col mstride: keep iff t-k >= 0
                        tri = pb[:, :2 * mstride].rearrange(
                            "p (j r t) -> p j r t",
                            j=2, t=128)[:, :, 0, :]
                        nc.gpsimd.affine_select(
                            out=tri,
                            in_=tri,
                            compare_op=ALU.is_ge,
                            fill=0.0,
                            base=0,
                            pattern=[[0, 2], [1, 128]],
                            channel_multiplier=-1,
                        )
                    pending.append(mk_avlb(
                        list(parts), st, pb, VC,
                        first=(gi == 0), last=(gi == ng - 1),
                        lb_first=(gi % 2 == 1)))
                flush(max(0, ng + 2 - popped))
                pending.append(mk_epiA(st))
                pending.append(mk_epiB(sl, p, st, pst))
                if ui in vins:
                    v_proj_slice(vins[ui])
                # drain 4 slice{2,3} slab scatters per early unit (slice 2
                # is first read at unit 4, slice 3 at unit 6)
                if ui < 4:
                    for _ in range(min(4, len(late_scatter))):
                        late_scatter.pop(0)()
            flush(len(pending))

    nc.compile()
    return nc


def kernel(query, key, values, Wq, bq, Wk, bk, Wv, bv, Wo, bo, mask):
    assert mask, "kernel compiled for causal attention (mask truthy)"
    query = np.asarray(query, np.float32)
    key = np.asarray(key, np.float32)
    values = np.asarray(values, np.float32)
    Wq_ = np.ascontiguousarray(np.asarray(Wq, np.float32)).astype(BF_NP)
    Wk_ = np.ascontiguousarray(np.asarray(Wk, np.float32)).astype(BF_NP)
    Wv_ = np.ascontiguousarray(np.asarray(Wv, np.float32)).astype(BF_NP)
    Wo_ = np.asarray(Wo, np.float32)
    bqT = np.ascontiguousarray(
        np.asarray(bq, np.float32).reshape(16, 128).T).astype(BF_NP)
    bkT = np.ascontiguousarray(
        np.asarray(bk, np.float32).reshape(16, 128).T).astype(BF_NP)
    bv_bc = np.ascontiguousarray(np.broadcast_to(
        np.asarray(bv, np.float32).reshape(1, 2048), (128, 2048))).astype(BF_NP)

    if "nc" not in _CACHE:
        _CACHE["nc"] = _build()
    nc = _CACHE["nc"]

    in_maps = []
    for c in range(NCORES):
        b = c // 4
        heads = [4 * (c % 4) + t for t in range(NSLICE)]
        qT = np.concatenate(
            [query[b, 128 * h:128 * (h + 1), :].T for h in heads], axis=1)
        kT = np.concatenate(
            [key[b, 128 * h:128 * (h + 1), :].T for h in heads], axis=1)
        vT = np.concatenate(
            [values[b, 128 * h:128 * (h + 1), :].T for h in heads], axis=1)
        Wo4 = np.concatenate(
            [Wo_[128 * h:128 * (h + 1), :] for h in heads], axis=1)
        in_maps.append({
            "qT": np.ascontiguousarray(qT).astype(BF_NP),
            "kT": np.ascontiguousarray(kT).astype(BF_NP),
            "vT": np.ascontiguousarray(vT).astype(BF_NP),
            "Wq": Wq_, "Wk": Wk_, "Wv": Wv_,
            "Wo4": np.ascontiguousarray(Wo4).astype(BF_NP),
            "bqT": bqT, "bkT": bkT, "bv_bc": bv_bc,
            "ones": _ONES,
        })

    _CACHE["last_in_maps"] = in_maps
    res = run_bass_kernel_spmd(nc, in_maps, list(range(NCORES)))
    out = np.empty((B, S, D), np.float32)
    bo_ = np.asarray(bo, np.float32)
    for b in range(B):
        part = res.results[4 * b]["partial"].astype(np.float32)
        for i in range(1, 4):
            part += res.results[4 * b + i]["partial"].astype(np.float32)
        out[b] = part.T + bo_
    return out


# revision 42
# speedup vs baseline: 1.0004x; 1.0004x over previous
"""Trainium2 Bass kernel for nn_MultiHeadAttention_45037027065972.

Head-parallel sharding: the reference's reshape `(B,S,H*D) -> (B,H,S,D)`
means head h of batch b only reads rows [128h, 128h+128) of the projection
inputs.  32 (b,h) slices are sharded 4-per-core across 8 cores (cores 0-3:
batch 0, cores 4-7: batch 1).  Each core projects its 4 slabs, runs full
S x S causal attention per slice in a transposed (k-major) layout, folds
the per-head output projection, and emits a per-core partial of
`sum_h out_h @ Wo_h` (shape [e=128, q=2048]).  The host unshard sums the
4 partials per batch, transposes, and adds bo.

v3 over the baseline:
  - ragged diagonal blocks: the k-chunk matmuls of the diagonal 512x512
    block only cover q >= 128j (scores, AV, lB and exp all skip ~15% of
    the attention work); causal masking shrinks to two [128,2,128]
    affine_selects per slice-panel on GPSIMD.
  - the Q^T/K^T slab eviction scatters two m-columns per call (the two
    projection matmuls share one PSUM tile), so the strided writes come
    in contiguous 8-byte pairs; the per-m bias rides along via a
    stride-0 broadcast AP.  (The biases do NOT cancel in softmax: the
    reshape makes them depend on s mod 16.)
  - ACT does nothing but exp during the attention phase; V-projection
    PSUM->SBUF copies and all epilogue work live on DVE, whose total
    load stays well under the PE's.
  - units run panels (3,sl),(2,sl) across slices, then (0,sl),(1,sl), so
    only two Wo accumulators are ever live; the V pipeline for slice
    sl+1 is interleaved right after the first unit touching slice sl.

Attention (S=2048, D=128), matmul operands bf16 (PSUM accumulate fp32):
  per slice scoresT[k,q] tiles = (K^T chunk stationary) @ (Q^T panel moving)
  P~ = exp(scoresT/sqrt(D)) on ACT (scores in [-9,9]: no running max);
  oT[d,q]  += V-chunk @ P~        (PSUM accumulation over k chunks)
  lB[*,q]  += ones128 @ P~        (row-sum broadcast across partitions)
  rb = 1/lB via DVE reciprocal_approx_fast; osbn = oT * rb (bf16)
  acc[e,q] += Wo_h^T @ osbn       (PSUM accumulation across the 4 slices)
"""

import sys
import math
from collections import deque

import numpy as np

for _p in ("/opt/trn_rl_repo", "/opt/pypackages"):
    if _p not in sys.path:
        sys.path.append(_p)

import ml_dtypes
import concourse.bacc as bacc
import concourse.mybir as mybir
import concourse.tile as tile
from concourse.bass_utils import run_bass_kernel_spmd

B, S, H, D = 2, 2048, 16, 128
NCORES = 8
NSLICE = 4            # (b,h) slices per core
PANEL = 512           # q panel width
NPANEL = S // PANEL   # 4
SCALE = 1.0 / math.sqrt(128.0)
F32 = mybir.dt.float32
BF16 = mybir.dt.bfloat16
F32R = mybir.dt.float32r
AF = mybir.ActivationFunctionType
ALU = mybir.AluOpType
BF_NP = ml_dtypes.bfloat16

_CACHE = {}
_ONES = np.ones((128, 128), BF_NP)


def _build():
    nc = bacc.Bacc(trn_type="TRN2", target_bir_lowering=False, debug=False)

    qT_d = nc.dram_tensor("qT", [128, NSLICE * 128], BF16, kind="ExternalInput")
    kT_d = nc.dram_tensor("kT", [128, NSLICE * 128], BF16, kind="ExternalInput")
    vT_d = nc.dram_tensor("vT", [128, NSLICE * 128], BF16, kind="ExternalInput")
    Wq_d = nc.dram_tensor("Wq", [128, 2048], BF16, kind="ExternalInput")
    Wk_d = nc.dram_tensor("Wk", [128, 2048], BF16, kind="ExternalInput")
    Wv_d = nc.dram_tensor("Wv", [128, 2048], BF16, kind="ExternalInput")
    Wo4_d = nc.dram_tensor("Wo4", [128, NSLICE * 128], BF16, kind="ExternalInput")
    bqT_d = nc.dram_tensor("bqT", [128, 16], BF16, kind="ExternalInput")
    bkT_d = nc.dram_tensor("bkT", [128, 16], BF16, kind="ExternalInput")
    bv_d = nc.dram_tensor("bv_bc", [128, 2048], BF16, kind="ExternalInput")
    ones_d = nc.dram_tensor("ones", [128, 128], BF16, kind="ExternalInput")
    out_d = nc.dram_tensor("partial", [128, S], BF16, kind="ExternalOutput")
    vscr_d = nc.dram_tensor("vscratch", [NSLICE, 128, 2048], BF16)

    with tile.TileContext(nc) as tc:
        with (
            tc.tile_pool(name="const", bufs=1) as const,
            tc.tile_pool(name="slab", bufs=1) as slab,
            tc.tile_pool(name="vslp", bufs=2) as vslp,
            tc.tile_pool(name="pbp", bufs=5) as pbp,
            tc.tile_pool(name="osbp", bufs=2) as osbp,
            tc.tile_pool(name="rbp", bufs=2) as rbp,
            tc.tile_pool(name="accp", bufs=2) as accp,
            tc.tile_pool(name="stgp", bufs=1) as stgp,
            tc.tile_pool(name="psS", bufs=2, space="PSUM") as psS,
            tc.tile_pool(name="psA", bufs=2, space="PSUM") as psA,
            tc.tile_pool(name="psO", bufs=2, space="PSUM") as psO,
        ):
            # ---- resident constants; DMA order = first-use order ----
            Wsb = {}
            xT = {}
            xdr = {"v": vT_d, "q": qT_d, "k": kT_d}
            wdr = {"v": Wv_d, "q": Wq_d, "k": Wk_d}
            for name in ("v", "q", "k"):
                t = const.tile([128, NSLICE * 128], BF16, tag=f"x{name}")
                xT[name] = t
                w = const.tile([128, 2048], BF16, tag=f"W{name}")
                Wsb[name] = w
            bqT = const.tile([128, 16], BF16, tag="bqT")
            bkT = const.tile([128, 16], BF16, tag="bkT")
            biasTT = {"q": bqT, "k": bkT}
            ones_sb = const.tile([128, 128], BF16, tag="ones")
            bv_sb = const.tile([128, 2048], BF16, tag="bv")
            wo4 = const.tile([128, NSLICE * 128], BF16, tag="wo4")
            dummy = const.tile([1, 16], F32, tag="dummy")

            # Force the exp table set to load NOW, while ACT is idle (the
            # Identity used by the ACT scatter pairs lives in every set, so
            # no second table switch before the first attention exp).
            nc.vector.memset(dummy[:], 1.0)
            nc.scalar.activation(dummy[:], dummy[:], AF.Exp)

            # Phase-1 DMAs (sync/gpsimd only: ACT must stay exp-only and
            # DVE is the scatter engine): V-pipeline inputs first (slice-0
            # V projection runs on the PE before the Q/K projection).
            nc.gpsimd.dma_start(out=Wsb["v"][:, :512], in_=wdr["v"][:, :512])
            nc.sync.dma_start(out=xT["v"][:], in_=xdr["v"][:])
            nc.sync.dma_start(out=Wsb["v"][:, 512:], in_=wdr["v"][:, 512:])
            nc.sync.dma_start(out=bv_sb[:], in_=bv_d[:])
            nc.gpsimd.dma_start(out=xT["k"][:], in_=xdr["k"][:])
            nc.sync.dma_start(out=xT["q"][:], in_=xdr["q"][:])
            nc.gpsimd.dma_start(out=Wsb["k"][:], in_=wdr["k"][:])
            nc.sync.dma_start(out=Wsb["q"][:], in_=wdr["q"][:])
            nc.gpsimd.dma_start(out=bqT[:], in_=bqT_d[:])
            nc.sync.dma_start(out=bkT[:], in_=bkT_d[:])
            nc.gpsimd.dma_start(out=ones_sb[:], in_=ones_d[:])
            nc.sync.dma_start(out=wo4[:], in_=Wo4_d[:])

            vch = slab.tile([128, NSLICE * 2048], BF16, tag="vch")

            def v_proj_slice(sl):
                # V projection for one slice: two [128,1024] PSUM tiles of
                # two 512-col matmuls each; the bias rides the DVE eviction
                # (STT add with the host-broadcast bv tile).  Then the DRAM
                # bounce into chunk layout
                # vch[16u+w, (i,d)] = vsl[8i+u, 128w+d] (dest [128,128]
                # iterates (16u+w, d) exactly as source [8,16,128] does).
                vsl = vslp.tile([128, 2048], BF16, tag="vsl")
                if sl == 0:
                    # slice 0 runs before attention: quarters staggered over
                    # the psA ring (idle until mid-attention) plus one psS
                    # slot, so neither the V0 matmuls nor the first
                    # projection tile ever wait on a DVE eviction
                    pools = [psA, psA, psS, psA]
                    tags = ["av", "av", "sc", "av"]
                    for qtr in range(4):
                        vq = pools[qtr].tile(
                            [128, 512], F32, tag=tags[qtr], name="vq")
                        nc.tensor.matmul(
                            vq[:],
                            lhsT=xT["v"][:, sl * 128:(sl + 1) * 128],
                            rhs=Wsb["v"][:, qtr * 512:(qtr + 1) * 512],
                            start=True, stop=True,
                        )
                        nc.vector.scalar_tensor_tensor(
                            vsl[:, qtr * 512:(qtr + 1) * 512], vq[:], 1.0,
                            bv_sb[:, qtr * 512:(qtr + 1) * 512],
                            op0=ALU.mult, op1=ALU.add)
                else:
                    for hf in range(2):
                        vq = psS.tile([128, 1024], F32, tag="sc", name="vq")
                        for qtr in (2 * hf, 2 * hf + 1):
                            nc.tensor.matmul(
                                vq[:, (qtr % 2) * 512:(qtr % 2) * 512 + 512],
                                lhsT=xT["v"][:, sl * 128:(sl + 1) * 128],
                                rhs=Wsb["v"][:, qtr * 512:(qtr + 1) * 512],
                                start=True, stop=True,
                            )
                        nc.vector.scalar_tensor_tensor(
                            vsl[:, hf * 1024:(hf + 1) * 1024], vq[:], 1.0,
                            bv_sb[:, hf * 1024:(hf + 1) * 1024],
                            op0=ALU.mult, op1=ALU.add)
                nc.sync.dma_start(out=vscr_d[sl], in_=vsl[:])
                nc.sync.dma_start(
                    out=vch[:, sl * 2048:(sl + 1) * 2048].rearrange(
                        "p (i d) -> p i d", i=16),
                    in_=vscr_d[sl].rearrange(
                        "(i u) (w d) -> (u w) i d", u=8, w=16),
                )

            v_proj_slice(0)

            # Q^T / K^T slabs in s' order: col (s, 16j + m).  Two-stage
            # eviction: ACT (idle pre-attention) copies each projection
            # m-pair PSUM tile to a persistent bf16 SBUF staging tile
            # (contiguous, fast); DVE then scatters m-pairs (contiguous
            # 4-byte write pairs at stride 32B) with the per-m bias via a
            # stride-0 broadcast AP.  Slices {0,1} scatter immediately so
            # attention can start as soon as the ACT stream drains; the
            # slice {2,3} scatters are deferred into the first attention
            # units (the staging tiles persist, so no PSUM lifetime issue).
            QKp = {}
            for name in ("q", "k"):
                dst = slab.tile([128, NSLICE * 2048], BF16, tag=f"{name}T")
                QKp[name] = dst
            late_scatter = []

            def mk_scatter(name, stg, mp, lo, hi):
                def go():
                    dst = QKp[name]
                    dview = dst[:].rearrange(
                        "p (sj w) -> p sj w",
                        w=16)[:, lo:hi, 2 * mp:2 * mp + 2]
                    src = stg[:].rearrange(
                        "p (w sj) -> p sj w", w=2)[:, lo:hi, :]
                    bias_bc = biasTT[name][:, 2 * mp:2 * mp + 2].rearrange(
                        "p (a w) -> p a w", a=1).broadcast_to(
                        [128, hi - lo, 2])
                    nc.vector.scalar_tensor_tensor(
                        dview, src, 1.0, bias_bc,
                        op0=ALU.mult, op1=ALU.add)
                return go

            for mp in range(8):
                for name in ("k", "q"):
                    pt = psS.tile([128, 1024], F32, tag="sc", name="pt")
                    for half in range(2):
                        m = 2 * mp + half
                        nc.tensor.matmul(
                            pt[:, half * 512:(half + 1) * 512],
                            lhsT=Wsb[name][:, m * 128:(m + 1) * 128],
                            rhs=xT[name][:],
                            start=True, stop=True,
                        )
                    stg = stgp.tile(
                        [128, 1024], BF16, tag=f"g{name}{mp}", name="stg")
                    nc.scalar.activation(stg[:], pt[:], AF.Copy)
                    mk_scatter(name, stg, mp, 0, 256)()
                    late_scatter.append(
                        mk_scatter(name, stg, mp, 256, 512))

            # ---- attention: panel-outer (desc), slice-inner ----
            # Deferred-emission queue: AV/lB for group g land two groups
            # later (hiding the exp + causal-mask latency), the epilogue
            # for a unit lands during the next unit's first groups.  PSUM
            # tiles are allocated lazily inside the deferred emitters so
            # pool rotation only ever reuses a tile whose readers are
            # already emitted.
            QT_all = QKp["q"]
            KT_all = QKp["k"]
            pending = deque()   # deferred emitters

            def flush(n):
                for _ in range(min(n, len(pending))):
                    pending.popleft()()

            def mk_avlb(parts, st, pb, VC, first, last, lb_first):
                # parts: list of (chunk_idx, pb_col_off, width, q_off).
                # lb_first alternates by group parity so the lB matmuls of
                # consecutive groups are adjacent and share the `ones`
                # stationary (one fewer LDWEIGHTS per group pair).
                def go():
                    if first:
                        st["oT"] = psA.tile([128, 512], F32, tag="av", name="oT")
                        st["lB"] = psA.tile([128, 512], F32, tag="av", name="lB")
                    oT, lB = st["oT"], st["lB"]
                    n = len(parts)

                    def av():
                        for pi, (ci, off, w, qo) in enumerate(parts):
                            nc.tensor.matmul(
                                oT[:, qo:512],
                                lhsT=VC[:, ci * 128:(ci + 1) * 128],
                                rhs=pb[:, off:off + w],
                                start=(first and pi == 0),
                                stop=(last and pi == n - 1),
                            )

                    def lb():
                        for pi, (ci, off, w, qo) in enumerate(parts):
                            nc.tensor.matmul(
                                lB[:, qo:512],
                                lhsT=ones_sb[:],
                                rhs=pb[:, off:off + w],
                                start=(first and pi == 0),
                                stop=(last and pi == n - 1),
                            )

                    if lb_first:
                        lb()
                        av()
                    else:
                        av()
                        lb()
                return go

            def mk_epiA(st):
                def go():
                    # normalize oT into bf16 osbn for the Wo fold
                    rb = rbp.tile([128, 512], F32, tag="rb")
                    nc.vector.reciprocal_approx_fast(
                        out=rb[:], in_=st["lB"][:])
                    osbn = osbp.tile([128, 512], BF16, tag="osbn")
                    nc.vector.tensor_tensor(
                        osbn[:], st["oT"][:], rb[:], ALU.mult)
                    st["osbn"] = osbn
                return go

            def mk_epiB(sl, p, st, pst):
                def go():
                    if sl == 0:
                        pst["acps"] = psO.tile([128, 512], F32, tag="acps", name="acps")
                    acps = pst["acps"]
                    nc.tensor.matmul(
                        acps[:],
                        lhsT=wo4[:, sl * 128:(sl + 1) * 128],
                        rhs=st["osbn"][:],
                        start=(sl == 0), stop=(sl == NSLICE - 1),
                    )
                    if sl == NSLICE - 1:
                        acc_sb = accp.tile([128, 512], BF16, tag="acc")
                        nc.vector.tensor_copy(acc_sb[:], acps[:])
                        nc.sync.dma_start(
                            out=out_d[:, p * 512:(p + 1) * 512],
                            in_=acc_sb[:])
                return go

            # unit order: the two long panels across all slices, then the
            # two short panels (only two Wo accumulators live at a time).
            units = []
            for sl in range(NSLICE):
                units += [(3, sl), (2, sl)]
            for sl in range(NSLICE):
                units += [(0, sl), (1, sl)]
            vins = {0: 1, 2: 2, 4: 3}  # after unit idx -> v_proj_slice(sl)
            psts = {p: {} for p in range(NPANEL)}
            for ui, (p, sl) in enumerate(units):
                pst = psts[p]
                VC = vch[:, sl * 2048:(sl + 1) * 2048]
                qt0 = sl * 2048 + p * 512
                st = {}
                # groups: full chunk pairs, then the ragged diagonal pairs.
                # Each entry: (sc_parts, mask_stride or None);
                # sc_parts: list of (chunk, sc_off, width, q_off)
                groups = []
                for g in range(2 * p):
                    groups.append((
                        [(2 * g, 0, 512, 0), (2 * g + 1, 512, 512, 0)],
                        None))
                groups.append((
                    [(4 * p, 0, 512, 0), (4 * p + 1, 512, 384, 128)],
                    512))
                groups.append((
                    [(4 * p + 2, 0, 256, 256), (4 * p + 3, 256, 128, 384)],
                    256))
                ng = len(groups)
                popped = 0
                for gi, (parts, mstride) in enumerate(groups):
                    sc = psS.tile([128, 1024], F32, tag="sc")
                    for ci, off, w, qo in parts:
                        nc.tensor.matmul(
                            sc[:, off:off + w],
                            lhsT=KT_all[:, sl * 2048 + ci * 128:
                                        sl * 2048 + (ci + 1) * 128],
                            rhs=QT_all[:, qt0 + qo:qt0 + 512],
                            start=True, stop=True,
                        )
                    # lag-3 software pipeline: consumers trail producers by
                    # three groups (pbp bufs=5 covers the pb lifetime)
                    n_pop = 1 if gi <= 1 else (2 if gi <= 3 else 1)
                    popped += n_pop
                    flush(n_pop)
                    width = parts[-1][1] + parts[-1][2]
                    pb = pbp.tile([128, 1024], BF16, tag="pb")
                    nc.scalar.activation(
                        pb[:, :width], sc[:, :width], AF.Exp, scale=SCALE)
                    if mstride is not None:
                        # zero where q < k in the two 128-wide triangle
                        # blocks at col 0 and col mstride: keep iff t-k >= 0
                        tri = pb[:, :2 * mstride].rearrange(
                            "p (j r t) -> p j r t",
                            j=2, t=128)[:, :, 0, :]
                        nc.gpsimd.affine_select(
                            out=tri,
                            in_=tri,
                            compare_op=ALU.is_ge,
                            fill=0.0,
                            base=0,
                            pattern=[[0, 2], [1, 128]],
                            channel_multiplier=-1,
                        )
                    pending.append(mk_avlb(
                        list(parts), st, pb, VC,
                        first=(gi == 0), last=(gi == ng - 1),
                        lb_first=(gi % 2 == 1)))
                flush(max(0, ng + 2 - popped))
                pending.append(mk_epiA(st))
                pending.append(mk_epiB(sl, p, st, pst))
                if ui in vins:
                    v_proj_slice(vins[ui])
                # drain 4 slice{2,3} slab scatters per early unit (slice 2
                # is first read at unit 4, slice 3 at unit 6)
                if ui < 4:
                    for _ in range(min(4, len(late_scatter))):
                        late_scatter.pop(0)()
            flush(len(pending))

    nc.compile()
    return nc


def kernel(query, key, values, Wq, bq, Wk, bk, Wv, bv, Wo, bo, mask):
    assert mask, "kernel compiled for causal attention (mask truthy)"
    query = np.asarray(query, np.float32)
    key = np.asarray(key, np.float32)
    values = np.asarray(values, np.float32)
    Wq_ = np.ascontiguousarray(np.asarray(Wq, np.float32)).astype(BF_NP)
    Wk_ = np.ascontiguousarray(np.asarray(Wk, np.float32)).astype(BF_NP)
    Wv_ = np.ascontiguousarray(np.asarray(Wv, np.float32)).astype(BF_NP)
    Wo_ = np.asarray(Wo, np.float32)
    bqT = np.ascontiguousarray(
        np.asarray(bq, np.float32).reshape(16, 128).T).astype(BF_NP)
    bkT = np.ascontiguousarray(
        np.asarray(bk, np.float32).reshape(16, 128).T).astype(BF_NP)
    bv_bc = np.ascontiguousarray(np.broadcast_to(
        np.asarray(bv, np.float32).reshape(1, 2048), (128, 2048))).astype(BF_NP)

    if "nc" not in _CACHE:
        _CACHE["nc"] = _build()
    nc = _CACHE["nc"]

    in_maps = []
    for c in range(NCORES):
        b = c // 4
        heads = [4 * (c % 4) + t for t in range(NSLICE)]
        qT = np.concatenate(
            [query[b, 128 * h:128 * (h + 1), :].T for h in heads], axis=1)
        kT = np.concatenate(
            [key[b, 128 * h:128 * (h + 1), :].T for h in heads], axis=1)
        vT = np.concatenate(
            [values[b, 128 * h:128 * (h + 1), :].T for h in heads], axis=1)
        Wo4 = np.concatenate(
            [Wo_[128 * h:128 * (h + 1), :] for h in heads], axis=1)
        in_maps.append({
            "qT": np.ascontiguousarray(qT).astype(BF_NP),
            "kT": np.ascontiguousarray(kT).astype(BF_NP),
            "vT": np.ascontiguousarray(vT).astype(BF_NP),
            "Wq": Wq_, "Wk": Wk_, "Wv": Wv_,
            "Wo4": np.ascontiguousarray(Wo4).astype(BF_NP),
            "bqT": bqT, "bkT": bkT, "bv_bc": bv_bc,
            "ones": _ONES,
        })

    _CACHE["last_in_maps"] = in_maps
    res = run_bass_kernel_spmd(nc, in_maps, list(range(NCORES)))
    out = np.empty((B, S, D), np.float32)
    bo_ = np.asarray(bo, np.float32)
    for b in range(B):
        part = res.results[4 * b]["partial"].astype(np.float32)
        for i in range(1, 4):
            part += res.results[4 * b + i]["partial"].astype(np.float32)
        out[b] = part.T + bo_
    return out
